# revision 1
# baseline (speedup 1.0000x reference)
"""2-layer GCN on 8 trn2 NeuronCores via Bass/Tile.

Design:
  - Nodes sharded contiguously across 8 cores (dst ownership).
  - Per layer: dense transform of the core's node shard (x @ W), AllGather
    of the transformed table, then edge aggregation:
      out[d] = sum_e norm_e * XW[src_e]  for dst d in shard
    done per window of 128 dst rows: dma_gather (int16, 4 sub-tables) of
    message rows + one-hot selection matrix built on DVE (is_equal vs iota)
    + PE matmul accumulating in PSUM.
  - Head: logits = emb @ Wc + bc, softmax, argmax via vector.max_index.

Host-side preprocessing computes degrees/norms, sorts edges by
(core, window, sub-table), pads each group to a multiple of 128, and packs
int16 gather indices in the dma_gather wrap layout.
"""
import sys
sys.path.insert(0, '/opt/trn_rl_repo')
import numpy as np

import concourse.bass as bass
import concourse.bacc as bacc
import concourse.tile as tile
import concourse.mybir as mybir
from concourse._compat import cdiv

P = 128
N_SUB = 4


class Cfg:
    def __init__(self, n_nodes, in_feats, hid, out_feats, n_cores=8):
        assert n_nodes % n_cores == 0
        self.n_nodes = n_nodes
        self.in_feats = in_feats
        self.hid = hid
        self.out_feats = out_feats
        self.n_cores = n_cores
        self.shard = n_nodes // n_cores
        self.nwin = cdiv(self.shard, P)
        assert n_nodes % N_SUB == 0
        self.subrows = n_nodes // N_SUB
        assert self.subrows <= 32767


def preprocess(x, edge_index, cfg):
    """Returns (per_core_inputs, shared_inputs, meta)."""
    n = cfg.n_nodes
    src = np.asarray(edge_index[0], dtype=np.int64)
    dst = np.asarray(edge_index[1], dtype=np.int64)
    deg = np.bincount(dst, minlength=n).astype(np.float32) + 1.0
    dinv = 1.0 / np.sqrt(deg)
    # self-loops are handled separately (contiguous rows, no gather):
    # contribution dinv[i]^2 * XW[i]
    a_src = src
    a_dst = dst
    norm = (dinv[a_src] * dinv[a_dst]).astype(np.float32)
    # per-core [128, nwin] tile of dinv^2 for the self-loop term
    dinv2 = (dinv * dinv).astype(np.float32)

    core = a_dst // cfg.shard
    local = a_dst % cfg.shard
    win = local // P
    rel = (local % P).astype(np.float32)
    sub = a_src // cfg.subrows
    i16 = (a_src % cfg.subrows).astype(np.int16)

    # group id: (core, win, sub)
    gid = (core * cfg.nwin + win) * N_SUB + sub
    order = np.argsort(gid, kind='stable')
    gid_s = gid[order]
    rel_s, norm_s, i16_s = rel[order], norm[order], i16[order]

    n_groups = cfg.n_cores * cfg.nwin * N_SUB
    counts = np.bincount(gid_s, minlength=n_groups).reshape(cfg.n_cores, cfg.nwin, N_SUB)
    # chunks per (win, sub): max over cores (program identical on all cores)
    nb_wt = np.maximum(cdiv_arr(counts, P).max(axis=0), 0)  # [nwin, N_SUB]
    nb_w = nb_wt.sum(axis=1)                                # [nwin]
    nbtot = int(nb_w.sum())
    nbmax = int(nb_w.max())
    # chunk column offsets, (w, t) raster order
    c0_wt = np.zeros((cfg.nwin, N_SUB), dtype=np.int64)
    flat = nb_wt.reshape(-1)
    c0_wt.reshape(-1)[1:] = np.cumsum(flat)[:-1]

    starts = np.zeros(n_groups + 1, dtype=np.int64)
    starts[1:] = np.cumsum(counts.reshape(-1))
    j_in_group = np.arange(len(gid_s)) - starts[gid_s]

    # per-edge padded coordinates
    w_s = (gid_s // N_SUB) % cfg.nwin
    t_s = gid_s % N_SUB
    c_s = (gid_s // (N_SUB * cfg.nwin))  # core
    chunk = c0_wt[w_s, t_s] + j_in_group // P
    part = j_in_group % P

    per_core = []
    for c in range(cfg.n_cores):
        m = c_s == c
        dstrel = np.full((P, nbtot), -1.0, dtype=np.float32)
        normv = np.zeros((P, nbtot), dtype=np.float32)
        idx16 = np.zeros((16, 8 * nbtot), dtype=np.int16)
        dstrel[part[m], chunk[m]] = rel_s[m]
        normv[part[m], chunk[m]] = norm_s[m]
        # wrap packing within each group: edge j -> [j%16, 8*C0 + j//16]
        jj = j_in_group[m]
        idx16[jj % 16, 8 * c0_wt[w_s[m], t_s[m]] + jj // 16] = i16_s[m]
        idx16_full = np.tile(idx16, (8, 1))
        xT = np.ascontiguousarray(x[c * cfg.shard:(c + 1) * cfg.shard, :].T)
        d2 = np.zeros((P, cfg.nwin), dtype=np.float32)
        d2_shard = dinv2[c * cfg.shard:(c + 1) * cfg.shard]
        pad = cfg.nwin * P - cfg.shard
        if pad:
            d2_shard = np.concatenate([d2_shard, np.zeros(pad, np.float32)])
        d2[:, :] = d2_shard.reshape(cfg.nwin, P).T
        per_core.append({
            "xT": xT, "dstrel": dstrel, "normv": normv, "idx16": idx16_full,
            "dinv2": d2,
        })

    iota_rep = np.tile(np.arange(P, dtype=np.float32), (P, max(nbmax, 1)))
    iota_rep = iota_rep.reshape(P, -1)[:, :max(nbmax, 1) * P].copy()
    # build per-window group list for the program
    win_groups = []
    for w in range(cfg.nwin):
        gs = []
        for t in range(N_SUB):
            if nb_wt[w, t] > 0:
                gs.append((t, int(c0_wt[w, t]), int(nb_wt[w, t])))
        win_groups.append(gs)
    meta = {"nbtot": nbtot, "nbmax": nbmax, "nb_w": nb_w.tolist(),
            "c0_w": [int(c0_wt[w, 0]) for w in range(cfg.nwin)],
            "win_groups": win_groups}
    shared = {"iota_rep": iota_rep}
    return per_core, shared, meta


def cdiv_arr(a, b):
    return (a + b - 1) // b


def build_nc(cfg, meta, enable_asserts=True, nq=1, reps=1):
    nwin, nbtot, nbmax = cfg.nwin, meta["nbtot"], meta["nbmax"]
    HID, OUT = cfg.hid, cfg.out_feats
    fp = mybir.dt.float32
    nc = bacc.Bacc("TRN2", target_bir_lowering=False, debug=False,
                   num_devices=cfg.n_cores, enable_asserts=enable_asserts,
                   num_swdge_queues=nq)
    qn = [0]

    # inputs
    xT_d = nc.dram_tensor("xT", [cfg.in_feats, cfg.shard], fp, kind="ExternalInput")
    dstrel_d = nc.dram_tensor("dstrel", [P, nbtot], fp, kind="ExternalInput")
    normv_d = nc.dram_tensor("normv", [P, nbtot], fp, kind="ExternalInput")
    idx16_d = nc.dram_tensor("idx16", [P, 8 * nbtot], mybir.dt.int16, kind="ExternalInput")
    iota_d = nc.dram_tensor("iota_rep", [P, max(nbmax, 1) * P], fp, kind="ExternalInput")
    W1_d = nc.dram_tensor("W1", [cfg.in_feats, HID], fp, kind="ExternalInput")
    W2_d = nc.dram_tensor("W2", [HID, HID], fp, kind="ExternalInput")
    Wc_d = nc.dram_tensor("Wc", [HID, OUT], fp, kind="ExternalInput")
    b1_d = nc.dram_tensor("b1", [P, HID], fp, kind="ExternalInput")
    b2_d = nc.dram_tensor("b2", [P, HID], fp, kind="ExternalInput")
    bc_d = nc.dram_tensor("bc", [P, OUT], fp, kind="ExternalInput")
    ident_d = nc.dram_tensor("ident", [P, P], fp, kind="ExternalInput")
    dinv2_d = nc.dram_tensor("dinv2", [P, nwin], fp, kind="ExternalInput")

    # outputs
    emb_o = nc.dram_tensor("emb", [cfg.shard, HID], fp, kind="ExternalOutput")
    logit_o = nc.dram_tensor("logits", [cfg.shard, OUT], fp, kind="ExternalOutput")
    soft_o = nc.dram_tensor("soft", [cfg.shard, OUT], fp, kind="ExternalOutput")
    hard_o = nc.dram_tensor("hard", [cfg.shard, 1], mybir.dt.int32, kind="ExternalOutput")

    ntile_in = cdiv(cfg.shard, P)  # tiles of 128 nodes for dense transforms

    with tile.TileContext(nc) as tc:
        import contextlib
        with contextlib.ExitStack() as ctx:
            const = ctx.enter_context(tc.tile_pool(name="const", bufs=1))
            xtp = ctx.enter_context(tc.tile_pool(name="xt", bufs=3))
            sbw = ctx.enter_context(tc.tile_pool(name="sbw", bufs=3))
            gp = ctx.enter_context(tc.tile_pool(name="gw", bufs=3))
            sp = ctx.enter_context(tc.tile_pool(name="sel", bufs=2))
            big = ctx.enter_context(tc.tile_pool(name="big", bufs=1))
            hp = ctx.enter_context(tc.tile_pool(name="head", bufs=2))
            ps_ag = ctx.enter_context(tc.tile_pool(name="ps_ag", bufs=2, space="PSUM"))
            ps_tr = ctx.enter_context(tc.tile_pool(name="ps_tr", bufs=2, space="PSUM"))
            ps_xw = ctx.enter_context(tc.tile_pool(name="ps_xw", bufs=2, space="PSUM"))
            ps_hd = ctx.enter_context(tc.tile_pool(name="ps_hd", bufs=2, space="PSUM"))
            dram = ctx.enter_context(tc.tile_pool(name="dram", bufs=1, space="DRAM"))

            # ---- load constants
            dstrel_t = const.tile([P, nbtot], fp, tag="dstrel")
            normv_t = const.tile([P, nbtot], fp, tag="normv")
            idx16_t = const.tile([P, 8 * nbtot], mybir.dt.int16, tag="idx16")
            iota_t = const.tile([P, max(nbmax, 1) * P], fp, tag="iota")
            W1_t = const.tile([cfg.in_feats, HID], fp, tag="W1")
            W2_t = const.tile([HID, HID], fp, tag="W2")
            Wc_t = const.tile([HID, OUT], fp, tag="Wc")
            b1_t = const.tile([P, HID], fp, tag="b1")
            b2_t = const.tile([P, HID], fp, tag="b2")
            bc_t = const.tile([P, OUT], fp, tag="bc")
            ident_t = const.tile([P, P], fp, tag="ident")
            dinv2_t = const.tile([P, nwin], fp, tag="dinv2")
            for t_, d_ in [(dstrel_t, dstrel_d), (normv_t, normv_d), (idx16_t, idx16_d),
                           (iota_t, iota_d), (W1_t, W1_d), (W2_t, W2_d), (Wc_t, Wc_d),
                           (b1_t, b1_d), (b2_t, b2_d), (bc_t, bc_d), (ident_t, ident_d), (dinv2_t, dinv2_d)]:
                nc.sync.dma_start(out=t_[:], in_=d_[:, :])

            h1_all = big.tile([P, nwin * HID], fp, tag="h1")
            emb_all = big.tile([P, nwin * HID], fp, tag="emb")

            def pipeline():
                cc1_in = dram.tile([cfg.shard, HID], fp, tag="cc1i")
                cc1_out = dram.tile([cfg.n_nodes, HID], fp, tag="cc1o")
                cc2_in = dram.tile([cfg.shard, HID], fp, tag="cc2i")
                cc2_out = dram.tile([cfg.n_nodes, HID], fp, tag="cc2o")
                def dense_transform(src_tile_fn, W_t, kdim, cc_in):
                    """cc_in[rows] = (X @ W) per 128-row tile; src_tile_fn(i) gives
                    lhsT tile [kdim, 128] in SBUF."""
                    for i in range(ntile_in):
                        rows = min(P, cfg.shard - i * P)
                        lhsT = src_tile_fn(i)
                        ps = ps_xw.tile([P, HID], fp, tag="ps_xw")
                        nc.tensor.matmul(out=ps[:], lhsT=lhsT, rhs=W_t[:kdim, :],
                                         start=True, stop=True)
                        ot = sbw.tile([P, HID], fp, tag="xw_out")
                        nc.vector.tensor_copy(out=ot[:], in_=ps[:])
                        nc.sync.dma_start(out=cc_in[i * P:i * P + rows, :], in_=ot[:rows, :])

                # ---- XW1: lhsT tiles direct from xT input
                def x_tile(i):
                    t_ = xtp.tile([cfg.in_feats, P], fp, tag="xT")
                    rows = min(P, cfg.shard - i * P)
                    nc.sync.dma_start(out=t_[:, :rows], in_=xT_d[:, i * P:i * P + rows])
                    return t_[:, :]

                dense_transform(x_tile, W1_t, cfg.in_feats, cc1_in)
                nc.gpsimd.collective_compute(
                    "AllGather", mybir.AluOpType.bypass,
                    replica_groups=[list(range(cfg.n_cores))],
                    ins=[cc1_in.opt()], outs=[cc1_out.opt()])

                def aggregate(cc_out, cc_in, w):
                    """Returns (psum, ownscaled) for window w: psum holds the
                    gathered-edge sum; ownscaled = dinv2 * XW[own rows] is the
                    self-loop term."""
                    rows = min(P, cfg.shard - w * P)
                    own = sbw.tile([P, HID], fp, tag="own")
                    nc.sync.dma_start(out=own[:rows, :], in_=cc_in[w * P:w * P + rows, :])
                    osc = sbw.tile([P, HID], fp, tag="osc")
                    nc.vector.tensor_scalar(
                        out=osc[:rows, :], in0=own[:rows, :],
                        scalar1=dinv2_t[:rows, w:w + 1], scalar2=None,
                        op0=mybir.AluOpType.mult)
                    gs = meta["win_groups"][w]
                    nb = meta["nb_w"][w]
                    c0 = meta["c0_w"][w]
                    gwin = gp.tile([P, max(nb, 1) * HID], fp, tag="gwin")
                    for (t, c0g, nbg) in gs:
                        nc.gpsimd.dma_gather(
                            gwin[:, (c0g - c0) * HID:(c0g - c0 + nbg) * HID]
                                .rearrange("p (b d) -> p b d", b=nbg),
                            cc_out[t * cfg.subrows:(t + 1) * cfg.subrows, :],
                            idx16_t[:, 8 * c0g:8 * (c0g + nbg)],
                            nbg * P, nbg * P, HID, single_packet=False,
                            queue_num=qn[0] % nq)
                        qn[0] += 1
                    # norm scale
                    nc.vector.tensor_tensor(
                        out=gwin[:, :nb * HID].rearrange("p (b d) -> p b d", b=nb),
                        in0=gwin[:, :nb * HID].rearrange("p (b d) -> p b d", b=nb),
                        in1=normv_t[:, c0:c0 + nb].to_broadcast([P, nb, HID]),
                        op=mybir.AluOpType.mult)
                    # selection matrix
                    S = sp.tile([P, max(nb, 1) * P], fp, tag="S")
                    nc.vector.tensor_tensor(
                        out=S[:, :nb * P].rearrange("p (b r) -> p b r", b=nb),
                        in0=dstrel_t[:, c0:c0 + nb].to_broadcast([P, nb, P]),
                        in1=iota_t[:, :nb * P].rearrange("p (b r) -> p b r", b=nb),
                        op=mybir.AluOpType.is_equal)
                    ps = ps_ag.tile([P, HID], fp, tag="ps_ag")
                    for k in range(nb):
                        nc.tensor.matmul(out=ps[:], lhsT=S[:, k * P:(k + 1) * P],
                                         rhs=gwin[:, k * HID:(k + 1) * HID],
                                         start=(k == 0), stop=(k == nb - 1))
                    return ps, osc

                # ---- layer 1 aggregation -> h1_all
                for w in range(nwin):
                    ps, osc = aggregate(cc1_out, cc1_in, w)
                    hs = h1_all[:, w * HID:(w + 1) * HID]
                    nc.vector.tensor_tensor(out=hs, in0=ps[:], in1=osc[:],
                                            op=mybir.AluOpType.add)
                    nc.vector.tensor_tensor(out=hs, in0=hs, in1=b1_t[:, :],
                                            op=mybir.AluOpType.add)
                    nc.scalar.activation(out=hs, in_=hs, func=mybir.ActivationFunctionType.Relu)

                # ---- XW2 from h1
                def h1_tile(i):
                    pt = ps_tr.tile([HID, P], fp, tag="ps_tr")
                    nc.tensor.transpose(out=pt[:], in_=h1_all[:, i * HID:(i + 1) * HID],
                                        identity=ident_t[:])
                    st = sbw.tile([HID, P], fp, tag="h1T")
                    nc.vector.tensor_copy(out=st[:], in_=pt[:])
                    return st[:, :]

                dense_transform(h1_tile, W2_t, HID, cc2_in)
                nc.gpsimd.collective_compute(
                    "AllGather", mybir.AluOpType.bypass,
                    replica_groups=[list(range(cfg.n_cores))],
                    ins=[cc2_in.opt()], outs=[cc2_out.opt()])

                # ---- layer 2 aggregation -> emb + head
                for w in range(nwin):
                    rows = min(P, cfg.shard - w * P)
                    ps, osc = aggregate(cc2_out, cc2_in, w)
                    es = emb_all[:, w * HID:(w + 1) * HID]
                    nc.vector.tensor_tensor(out=es, in0=ps[:], in1=osc[:],
                                            op=mybir.AluOpType.add)
                    nc.vector.tensor_tensor(out=es, in0=es, in1=b2_t[:, :],
                                            op=mybir.AluOpType.add)
                    nc.sync.dma_start(out=emb_o[w * P:w * P + rows, :], in_=es[:rows, :])

                    # head: logits
                    pt = ps_tr.tile([HID, P], fp, tag="ps_tr")
                    nc.tensor.transpose(out=pt[:], in_=es, identity=ident_t[:])
                    eT = sbw.tile([HID, P], fp, tag="h1T")
                    nc.vector.tensor_copy(out=eT[:], in_=pt[:])
                    ph = ps_hd.tile([P, OUT], fp, tag="ps_hd")
                    nc.tensor.matmul(out=ph[:], lhsT=eT[:, :], rhs=Wc_t[:, :],
                                     start=True, stop=True)
                    lg = hp.tile([P, OUT], fp, tag="lg")
                    nc.vector.tensor_tensor(out=lg[:], in0=ph[:], in1=bc_t[:, :],
                                            op=mybir.AluOpType.add)
                    nc.sync.dma_start(out=logit_o[w * P:w * P + rows, :], in_=lg[:rows, :])
                    # softmax
                    nm = hp.tile([P, 1], fp, tag="nm")
                    nc.vector.tensor_reduce(out=nm[:], in_=lg[:], op=mybir.AluOpType.max,
                                            axis=mybir.AxisListType.X, negate=True)
                    ex = hp.tile([P, OUT], fp, tag="ex")
                    nc.scalar.activation(out=ex[:], in_=lg[:],
                                         func=mybir.ActivationFunctionType.Exp,
                                         bias=nm[:, :], scale=1.0)
                    sm = hp.tile([P, 1], fp, tag="sm")
                    nc.vector.tensor_reduce(out=sm[:], in_=ex[:], op=mybir.AluOpType.add,
                                            axis=mybir.AxisListType.X)
                    rc = hp.tile([P, 1], fp, tag="rc")
                    nc.vector.reciprocal(out=rc[:], in_=sm[:])
                    st = hp.tile([P, OUT], fp, tag="st")
                    nc.scalar.activation(out=st[:], in_=ex[:],
                                         func=mybir.ActivationFunctionType.Copy,
                                         bias=0.0, scale=rc[:, :])
                    nc.sync.dma_start(out=soft_o[w * P:w * P + rows, :], in_=st[:rows, :])
                    # argmax
                    m8 = hp.tile([P, 8], fp, tag="m8")
                    i8 = hp.tile([P, 8], mybir.dt.uint32, tag="i8")
                    nc.vector.max(out=m8[:], in_=st[:])
                    nc.vector.max_index(out=i8[:], in_max=m8[:], in_values=st[:])
                    hd = hp.tile([P, 1], mybir.dt.int32, tag="hd")
                    nc.vector.tensor_copy(out=hd[:], in_=i8[:, 0:1])
                    nc.sync.dma_start(out=hard_o[w * P:w * P + rows, :], in_=hd[:rows, :])


            for _rep in range(reps):
                pipeline()

    nc.compile()
    return nc


def make_in_maps(cfg, per_core, shared, W1, b1, W2, b2, Wc, bc):
    ident = np.eye(P, dtype=np.float32)
    maps = []
    for c in range(cfg.n_cores):
        pc = per_core[c]
        maps.append({
            "xT": pc["xT"], "dstrel": pc["dstrel"], "normv": pc["normv"],
            "idx16": pc["idx16"], "iota_rep": shared["iota_rep"],
            "W1": np.asarray(W1, np.float32), "W2": np.asarray(W2, np.float32),
            "Wc": np.asarray(Wc, np.float32),
            "b1": np.tile(np.asarray(b1, np.float32).reshape(1, -1), (128, 1)),
            "b2": np.tile(np.asarray(b2, np.float32).reshape(1, -1), (128, 1)),
            "bc": np.tile(np.asarray(bc, np.float32).reshape(1, -1), (128, 1)),
            "ident": ident, "dinv2": pc["dinv2"],
        })
    return maps


def assemble(cfg, results):
    logits = np.concatenate([r["logits"] for r in results], axis=0)
    emb = np.concatenate([r["emb"] for r in results], axis=0)
    soft = np.concatenate([r["soft"] for r in results], axis=0)
    hard = np.concatenate([r["hard"] for r in results], axis=0)[:, 0]
    return logits, emb, soft, hard


# ---------------------------------------------------------------------------
# Harness entry point: kernel(**inputs) -> (logits, embedding, soft_label,
# hard_label), matching reference.reference(). Self-contained: shards the
# inputs across 8 NeuronCores, compiles and runs the Bass kernel via
# run_bass_kernel_spmd, and reassembles full outputs.
# ---------------------------------------------------------------------------
from concourse.bass_utils import run_bass_kernel_spmd

_CACHE = {}


def kernel(x, edge_index, W1, b1, W2, b2, Wc, bc):
    x = np.asarray(x, dtype=np.float32)
    edge_index = np.asarray(edge_index)
    W1 = np.asarray(W1, dtype=np.float32)
    b1 = np.asarray(b1, dtype=np.float32)
    W2 = np.asarray(W2, dtype=np.float32)
    b2 = np.asarray(b2, dtype=np.float32)
    Wc = np.asarray(Wc, dtype=np.float32)
    bc = np.asarray(bc, dtype=np.float32)

    cfg = Cfg(x.shape[0], x.shape[1], W2.shape[0], Wc.shape[1])
    per_core, shared, meta = preprocess(x, edge_index, cfg)
    nc = build_nc(cfg, meta, nq=4)
    in_maps = make_in_maps(cfg, per_core, shared, W1, b1, W2, b2, Wc, bc)
    res = run_bass_kernel_spmd(nc, in_maps, core_ids=list(range(cfg.n_cores)))
    logits, emb, soft, hard = assemble(cfg, res.results)
    return (logits, emb, soft, hard.astype(np.int64))
